# revision 17
# baseline (speedup 1.0000x reference)
"""Trainium2 Bass kernel for the additive-attention layer (B=32, T=8192, V=A=128).

Strategy: data-parallel over batch across 8 NeuronCores (4 batches/core).
Per batch on-device:
  phase 1 (scores):  featT[a,t] = Wk^T encT + Kc (x) cov  (PE, PSUM-accumulated)
                     tanhT = tanh(featT + bias[a])         (ACT, fused bias)
                     e[t]  = v . tanhT[:, t]               (PE, t-partitioned out)
  softmax:           max via free-reduce + partition butterfly, exp via ACT
                     (fused -max bias), mask-mult + row-sum fused on DVE,
                     1/sum broadcast via tiny PE matmuls
  phase 2 (context): ctx[v] = sum_t attn[t] enc[t,v]       (PE, 64 accumulating
                     matmuls with attn columns as stationary operand)
The host pre-transposes encoder_state to [B, V, T] for the scores pass and
reads [B, T, V] natural for the context pass (fp32 needs enc in both layouts;
on-chip fp32 transpose paths are slower than a second HBM read).
e/attn use a "jp" layout: value for t = j*128 + p lives at [p, j].
"""

import sys

for _p in ("/opt/trn_rl_repo",):
    if _p not in sys.path:
        sys.path.insert(0, _p)

import numpy as np
import concourse.bass as bass
import concourse.bacc as bacc
import concourse.tile as tile
import concourse.mybir as mybir
from concourse.bass_utils import run_bass_kernel_spmd

F32 = mybir.dt.float32
B, T, V = 32, 8192, 128
NCORES = 8
BS = B // NCORES          # batches per core
NJ = T // 128             # 64 e-columns per batch
CH = 1024                 # phase-1 chunk length along t
NCHUNK = T // CH

_BUILD_CACHE = {}


def _build(rep=1):
    if rep in _BUILD_CACHE:
        return _BUILD_CACHE[rep]
    nc = bacc.Bacc("TRN2", target_bir_lowering=False, debug=False, num_devices=NCORES)
    d_encT = nc.dram_tensor("encT", [BS, V, T], F32, kind="ExternalInput").ap()
    d_enc = nc.dram_tensor("enc", [BS, T, V], F32, kind="ExternalInput").ap()
    d_covr = nc.dram_tensor("covrow", [BS, T], F32, kind="ExternalInput").ap()
    d_covjp = nc.dram_tensor("covjp", [BS, 128, NJ], F32, kind="ExternalInput").ap()
    d_maskjp = nc.dram_tensor("maskjp", [BS, 128, NJ], F32, kind="ExternalInput").ap()
    d_biasT = nc.dram_tensor("biasT", [128, BS], F32, kind="ExternalInput").ap()
    d_wk = nc.dram_tensor("wk", [V, V], F32, kind="ExternalInput").ap()
    d_kc = nc.dram_tensor("kcm", [BS, BS * V], F32, kind="ExternalInput").ap()
    d_v = nc.dram_tensor("vcol", [V, 1], F32, kind="ExternalInput").ap()
    d_ones_col = nc.dram_tensor("ones_col", [128, 1], F32, kind="ExternalInput").ap()
    d_ones_row = nc.dram_tensor("ones_row", [1, 128], F32, kind="ExternalInput").ap()
    d_neg_row = nc.dram_tensor("negones_row", [1, 128], F32, kind="ExternalInput").ap()
    d_ident = nc.dram_tensor("ident", [128, 128], F32, kind="ExternalInput").ap()
    d_attn = nc.dram_tensor("attn_jp", [BS, 128, NJ], F32, kind="ExternalOutput").ap()
    d_covo = nc.dram_tensor("covout_jp", [BS, 128, NJ], F32, kind="ExternalOutput").ap()
    d_ctx = nc.dram_tensor("ctx", [BS, V], F32, kind="ExternalOutput").ap()

    AF = mybir.ActivationFunctionType
    OP = mybir.AluOpType

    with tile.TileContext(nc) as tc:
        with (
            tc.tile_pool(name="singles", bufs=1) as singles,
            tc.tile_pool(name="slabT", bufs=2) as slabT_pool,
            tc.tile_pool(name="slabN", bufs=2) as slabN_pool,
            tc.tile_pool(name="tanh", bufs=2) as tanh_pool,
            tc.tile_pool(name="work", bufs=2) as work_pool,
            tc.tile_pool(name="small", bufs=2) as small_pool,
            tc.tile_pool(name="featps", bufs=2, space=bass.MemorySpace.PSUM) as featps,
            tc.tile_pool(name="eps", bufs=2, space=bass.MemorySpace.PSUM) as eps_pool,
            tc.tile_pool(name="ctxps", bufs=1, space=bass.MemorySpace.PSUM) as ctxps,
            tc.tile_pool(name="smallps", bufs=1, space=bass.MemorySpace.PSUM) as smallps,
        ):
            wk_sb = singles.tile([V, V], F32)
            nc.sync.dma_start(out=wk_sb[:], in_=d_wk[:])
            kc_sb = singles.tile([BS, BS * V], F32)
            nc.sync.dma_start(out=kc_sb[:], in_=d_kc[:])
            v_sb = singles.tile([V, 1], F32)
            nc.sync.dma_start(out=v_sb[:], in_=d_v[:])
            onc_sb = singles.tile([128, 1], F32)
            nc.sync.dma_start(out=onc_sb[:], in_=d_ones_col[:])
            onr_sb = singles.tile([1, 128], F32)
            nc.sync.dma_start(out=onr_sb[:], in_=d_ones_row[:])
            ngr_sb = singles.tile([1, 128], F32)
            nc.sync.dma_start(out=ngr_sb[:], in_=d_neg_row[:])
            ident_sb = singles.tile([128, 128], F32)
            nc.sync.dma_start(out=ident_sb[:], in_=d_ident[:])
            biasT_sb = singles.tile([128, BS], F32)
            nc.sync.dma_start(out=biasT_sb[:], in_=d_biasT[:])
            maskjp_sb = singles.tile([128, BS, NJ], F32)
            nc.sync.dma_start(out=maskjp_sb[:], in_=d_maskjp.rearrange("b p j -> p b j"))
            covjp_sb = singles.tile([128, BS, NJ], F32)
            nc.sync.dma_start(out=covjp_sb[:], in_=d_covjp.rearrange("b p j -> p b j"))
            covr_all = singles.tile([BS, T], F32)
            nc.sync.dma_start(out=covr_all[:], in_=d_covr[:])

            for _ in range(rep):
                for b in range(BS):
                    encT_slab = slabT_pool.tile([V, T], F32, tag="encT")
                    nc.sync.dma_start(out=encT_slab[:], in_=d_encT[b])
                    enc_slab = slabN_pool.tile([128, NJ, V], F32, tag="encN")
                    nc.sync.dma_start(
                        out=enc_slab[:],
                        in_=d_enc[b].rearrange("(j p) v -> p j v", p=128),
                    )
                    kcb_sb = kc_sb[:, b * V : (b + 1) * V]

                    # phase 1: scores
                    e_ps = eps_pool.tile([128, NJ], F32, tag="e")
                    for c in range(NCHUNK):
                        feat = featps.tile([V, CH], F32, tag="feat")
                        t0 = c * CH
                        for h in range(CH // 512):
                            sl = slice(h * 512, (h + 1) * 512)
                            nc.tensor.matmul(
                                feat[:, sl], wk_sb[:],
                                encT_slab[:, t0 + h * 512 : t0 + (h + 1) * 512],
                                start=True, stop=False,
                            )
                        for h in range(CH // 512):
                            sl = slice(h * 512, (h + 1) * 512)
                            nc.tensor.matmul(
                                feat[:, sl], kcb_sb,
                                covr_all[:, t0 + h * 512 : t0 + (h + 1) * 512],
                                start=False, stop=True,
                            )
                        tanh_sb = tanh_pool.tile([V, CH], F32, tag="tanh")
                        nc.scalar.activation(
                            tanh_sb[:], feat[:], AF.Tanh,
                            bias=biasT_sb[:, b : b + 1], scale=1.0,
                        )
                        for j in range(CH // 128):
                            jj = c * (CH // 128) + j
                            nc.tensor.matmul(
                                e_ps[:, jj : jj + 1],
                                tanh_sb[:, j * 128 : (j + 1) * 128],
                                v_sb[:], start=True, stop=True,
                            )

                    # masked softmax over T with renormalization
                    mp = small_pool.tile([128, 1], F32, tag="mp")
                    nc.vector.reduce_max(mp[:], e_ps[:], axis=mybir.AxisListType.X)
                    mpT_ps = smallps.tile([1, 128], F32, tag="sp")
                    nc.tensor.matmul(mpT_ps[:], mp[:], ident_sb[:], start=True, stop=True)
                    mglob = small_pool.tile([1, 1], F32, tag="mglob")
                    nc.vector.reduce_max(mglob[:], mpT_ps[:], axis=mybir.AxisListType.X)
                    negm_ps = smallps.tile([128, 1], F32, tag="sp")
                    nc.tensor.matmul(negm_ps[:], ngr_sb[:], mglob[:], start=True, stop=True)
                    negm = small_pool.tile([128, 1], F32, tag="negm")
                    nc.vector.tensor_copy(negm[:], negm_ps[:])
                    w_sb = work_pool.tile([128, NJ], F32, tag="w")
                    nc.scalar.activation(w_sb[:], e_ps[:], AF.Exp, bias=negm[:], scale=1.0)
                    wm_sb = work_pool.tile([128, NJ], F32, tag="wm")
                    spart = small_pool.tile([128, 1], F32, tag="spart")
                    nc.vector.tensor_tensor(
                        out=wm_sb[:], in0=w_sb[:], in1=maskjp_sb[:, b, :], op=OP.mult
                    )
                    nc.vector.tensor_reduce(
                        spart[:], wm_sb[:], axis=mybir.AxisListType.X, op=OP.add
                    )
                    ssum_ps = smallps.tile([1, 1], F32, tag="sp")
                    nc.tensor.matmul(ssum_ps[:], spart[:], onc_sb[:], start=True, stop=True)
                    r_sb = small_pool.tile([1, 1], F32, tag="r")
                    nc.vector.reciprocal(r_sb[:], ssum_ps[:])
                    rb_ps = smallps.tile([128, 1], F32, tag="sp")
                    nc.tensor.matmul(rb_ps[:], onr_sb[:], r_sb[:], start=True, stop=True)
                    rb = small_pool.tile([128, 1], F32, tag="rb")
                    nc.vector.tensor_copy(rb[:], rb_ps[:])
                    attn_sb = work_pool.tile([128, NJ], F32, tag="attn")
                    nc.vector.tensor_scalar_mul(attn_sb[:], wm_sb[:], rb[:])
                    covo_sb = work_pool.tile([128, NJ], F32, tag="covo")
                    nc.vector.tensor_tensor(
                        out=covo_sb[:], in0=attn_sb[:], in1=covjp_sb[:, b, :], op=OP.add
                    )
                    nc.sync.dma_start(out=d_attn[b], in_=attn_sb[:])
                    nc.sync.dma_start(out=d_covo[b], in_=covo_sb[:])

                    # phase 2: context
                    ctx_ps = ctxps.tile([1, V], F32, tag="ctx")
                    for j in range(NJ):
                        nc.tensor.matmul(
                            ctx_ps[:], attn_sb[:, j : j + 1], enc_slab[:, j, :],
                            start=(j == 0), stop=(j == NJ - 1),
                        )
                    ctx_sb = small_pool.tile([1, V], F32, tag="ctxsb")
                    nc.scalar.copy(ctx_sb[:], ctx_ps[:])
                    nc.sync.dma_start(out=d_ctx[b : b + 1, :], in_=ctx_sb[:])

    nc.compile()
    _BUILD_CACHE[rep] = nc
    return nc


def _prepare_in_maps(encoder_state, decoder_state, input_mask, coverage,
                     Wk_enc, b_enc, W_dec, b_dec, Kc_cov, b_cov, v):
    enc = np.ascontiguousarray(np.asarray(encoder_state, dtype=np.float32))
    dec = np.asarray(decoder_state, dtype=np.float32)
    mask = np.asarray(input_mask, dtype=np.float32)
    cov = np.asarray(coverage, dtype=np.float32)[:, :, 0, 0]
    Wk = np.ascontiguousarray(np.asarray(Wk_enc, dtype=np.float32))
    vv = np.asarray(v, dtype=np.float32)

    dec_feat = dec @ np.asarray(W_dec, np.float32) + np.asarray(b_dec, np.float32)
    bias_ba = dec_feat + np.asarray(b_enc, np.float32) + np.asarray(b_cov, np.float32)

    encT = np.ascontiguousarray(enc.transpose(0, 2, 1))
    mask_jp = np.ascontiguousarray(mask.reshape(B, NJ, 128).transpose(0, 2, 1))
    cov_jp = np.ascontiguousarray(cov.reshape(B, NJ, 128).transpose(0, 2, 1))

    ones_col = np.ones((128, 1), np.float32)
    ones_row = np.ones((1, 128), np.float32)
    neg_row = -np.ones((1, 128), np.float32)
    ident = np.eye(128, dtype=np.float32)
    kc1 = np.asarray(Kc_cov, np.float32).reshape(V)
    kcm = np.zeros((BS, BS * V), np.float32)
    for bb in range(BS):
        kcm[bb, bb * V : (bb + 1) * V] = kc1
    vcol = np.ascontiguousarray(vv.reshape(V, 1))

    in_maps = []
    for i in range(NCORES):
        s = slice(BS * i, BS * (i + 1))
        in_maps.append({
            "encT": encT[s],
            "enc": enc[s],
            "covrow": np.ascontiguousarray(cov[s]),
            "covjp": cov_jp[s],
            "maskjp": mask_jp[s],
            "biasT": np.ascontiguousarray(bias_ba[s].T),
            "wk": Wk,
            "kcm": kcm,
            "vcol": vcol,
            "ones_col": ones_col,
            "ones_row": ones_row,
            "negones_row": neg_row,
            "ident": ident,
        })
    return in_maps, cov


def _assemble(results, coverage_dtype_shape_ref):
    attn_jp = np.concatenate([r["attn_jp"] for r in results], axis=0)   # [B,128,NJ]
    covo_jp = np.concatenate([r["covout_jp"] for r in results], axis=0)
    ctx = np.concatenate([r["ctx"] for r in results], axis=0)           # [B,V]
    attn = np.ascontiguousarray(attn_jp.transpose(0, 2, 1)).reshape(B, T)
    covo = np.ascontiguousarray(covo_jp.transpose(0, 2, 1)).reshape(B, T, 1, 1)
    return ctx.astype(np.float32), attn.astype(np.float32), covo.astype(np.float32)


def kernel(**inputs):
    nc = _build(rep=1)
    in_maps, _ = _prepare_in_maps(**inputs)
    res = run_bass_kernel_spmd(nc, in_maps, list(range(NCORES)))
    return _assemble(res.results, None)
